# revision 4
# baseline (speedup 1.0000x reference)
"""Trainium2 kernel for nn_Conv2Seq: conv frontend + attention decoder + BiLSTM stack.

Strategy (per spec sharding hint): pure data parallel. Batch dim (64) is sharded
8 ways across the 8 NeuronCores; all weights are replicated. The 60-step
recurrences run sequentially on each core over its local batch shard.

Implementation runs on the NeuronCores through the PJRT backend with a
jax.pmap SPMD program (one program per core, batch-sharded inputs,
broadcast weights). Shapes are hardcoded from the problem spec:
  x [64, 4096, 64] f32 -> out [64, 60, 128] f32
"""

import numpy as np
import jax
import jax.numpy as jnp

T_DEC = 60
N_CORES = 8
B, L, D_IN, H, D_OUT = 64, 4096, 64, 256, 128


def _conv1d(x, w, b):
    # x: [B, C_in, L], w: [C_out, C_in, K] -> [B, C_out, L-K+1] (valid conv)
    y = jax.lax.conv_general_dilated(
        x, w, window_strides=(1,), padding="VALID",
        dimension_numbers=("NCH", "OIH", "NCH"),
    )
    return y + b[None, :, None]


def _lstm_cell(x, h, c, wih, whh, bih, bhh):
    g = x @ wih.T + h @ whh.T + (bih + bhh)
    i, f, gg, o = jnp.split(g, 4, axis=-1)
    c = jax.nn.sigmoid(f) * c + jax.nn.sigmoid(i) * jnp.tanh(gg)
    h = jax.nn.sigmoid(o) * jnp.tanh(c)
    return h, c


def _lstm_seq(xs, wih, whh, bih, bhh, reverse=False):
    Bl = xs.shape[0]
    xs_t = jnp.swapaxes(xs, 0, 1)
    if reverse:
        xs_t = xs_t[::-1]

    def step(carry, x):
        h, c = carry
        h, c = _lstm_cell(x, h, c, wih, whh, bih, bhh)
        return (h, c), h

    init = (jnp.zeros((Bl, H), xs.dtype), jnp.zeros((Bl, H), xs.dtype))
    _, hs = jax.lax.scan(step, init, xs_t)
    if reverse:
        hs = hs[::-1]
    return jnp.swapaxes(hs, 0, 1)


def _forward(x, conv_w1, conv_b1, conv_w2, conv_b2, conv_w3, conv_b3,
             attn_w, attn_b, dec1_wih, dec1_whh, dec1_bih, dec1_bhh,
             dec2_wih0, dec2_whh0, dec2_bih0, dec2_bhh0,
             dec2_wih1, dec2_whh1, dec2_bih1, dec2_bhh1, out_w, out_b):
    # x here is the per-core shard [B/8, L, D_in]
    h = jnp.swapaxes(x, 1, 2)
    h = jax.nn.relu(_conv1d(h, conv_w1, conv_b1))
    h = jax.nn.relu(_conv1d(h, conv_w2, conv_b2))
    h = jax.nn.relu(_conv1d(h, conv_w3, conv_b3))
    enc = jnp.swapaxes(h, 1, 2)  # [Bl, Lp, 32]
    Bl = x.shape[0]
    Lp = enc.shape[1]

    # Loop-invariant attention layouts. All per-step math is plain 2D matmuls
    # (the neuronx compiler ICEs on batched dots / 3D reduces inside loops).
    enc_cat = jnp.transpose(enc, (1, 0, 2)).reshape(Lp, Bl * 32)  # [Lp, Bl*32]
    enc_catT = enc_cat.T  # [Bl*32, Lp]
    eye_b = jnp.eye(Bl, dtype=x.dtype)

    # Fold attn into a per-step query: align = (h @ attn_w) . enc + h . attn_b.
    # This avoids materializing keys_t [Bl, H, Lp] entirely.
    def dec1(wih, whh, bih, bhh):
        def step(carry, _):
            hh, cc = carry
            q = hh @ attn_w            # [Bl, 32]
            s = hh @ attn_b            # [Bl]
            q_bd = (q[:, :, None] * eye_b[:, None, :]).reshape(Bl * 32, Bl)
            align = enc_cat @ q_bd + s[None, :]      # [Lp, Bl]
            m = jnp.max(align, axis=0)               # [Bl]
            e = jnp.exp(align - m[None, :])          # [Lp, Bl]
            z = jnp.sum(e, axis=0)                   # [Bl]
            ctx_pre = enc_catT @ e                   # [Bl*32, Bl]
            ctx = (ctx_pre.reshape(Bl, 32, Bl) * eye_b[:, None, :]).sum(-1)
            ctx = ctx / z[:, None]
            hh, cc = _lstm_cell(ctx, hh, cc, wih, whh, bih, bhh)
            return (hh, cc), hh

        init = (jnp.zeros((Bl, H), x.dtype), jnp.zeros((Bl, H), x.dtype))
        _, hs = jax.lax.scan(step, init, None, length=T_DEC)
        return jnp.swapaxes(hs, 0, 1)

    fwd = dec1(dec1_wih[0], dec1_whh[0], dec1_bih[0], dec1_bhh[0])
    bkwd = dec1(dec1_wih[1], dec1_whh[1], dec1_bih[1], dec1_bhh[1])[:, ::-1]
    d2in = jnp.concatenate([fwd, bkwd], axis=-1)
    l0 = jnp.concatenate([
        _lstm_seq(d2in, dec2_wih0[0], dec2_whh0[0], dec2_bih0[0], dec2_bhh0[0]),
        _lstm_seq(d2in, dec2_wih0[1], dec2_whh0[1], dec2_bih0[1], dec2_bhh0[1], reverse=True)], axis=-1)
    l1 = jnp.concatenate([
        _lstm_seq(l0, dec2_wih1[0], dec2_whh1[0], dec2_bih1[0], dec2_bhh1[0]),
        _lstm_seq(l0, dec2_wih1[1], dec2_whh1[1], dec2_bih1[1], dec2_bhh1[1], reverse=True)], axis=-1)
    return l1 @ out_w.T + out_b


_INPUT_ORDER = [
    "x", "conv_w1", "conv_b1", "conv_w2", "conv_b2", "conv_w3", "conv_b3",
    "attn_w", "attn_b", "dec1_wih", "dec1_whh", "dec1_bih", "dec1_bhh",
    "dec2_wih0", "dec2_whh0", "dec2_bih0", "dec2_bhh0",
    "dec2_wih1", "dec2_whh1", "dec2_bih1", "dec2_bhh1", "out_w", "out_b",
]

_pmapped = None


def _get_pmapped():
    global _pmapped
    if _pmapped is None:
        devs = jax.devices()[:N_CORES]
        _pmapped = jax.pmap(
            _forward,
            in_axes=(0,) + (None,) * 22,
            devices=devs,
        )
    return _pmapped


def kernel(**inputs):
    fn = _get_pmapped()
    args = [inputs[k] for k in _INPUT_ORDER]
    x = np.ascontiguousarray(args[0], dtype=np.float32)
    xs = x.reshape(N_CORES, B // N_CORES, L, D_IN)
    ws = [jnp.asarray(np.asarray(a, dtype=np.float32)) for a in args[1:]]
    out = fn(jnp.asarray(xs), *ws)  # [8, 8, 60, 128]
    out = np.asarray(jax.device_get(out), dtype=np.float32)
    return out.reshape(B, T_DEC, D_OUT)
